# revision 2
# baseline (speedup 1.0000x reference)
"""EntityLinker Trainium2 kernel (8 NeuronCores, entity-dim tensor parallel).

Per-core plan (tokens T=1024, H=768, D=256, E_local=6272 of E_pad=50176):
  - load hidden/W/md_W/b/md_b (replicated) + this core's entity shard
  - PE-transpose hidden -> hT[6][128,1024] (f32r), W -> WT[6][128,256] (f32r)
  - proj = tanh(hidden @ W.T + b) via f32r matmuls, [tok,D] layout
  - token norms via ACT Square+accum; proj_n = proj * inv_norm (DVE)
  - PE-transpose proj_n -> projT[2][128,1024] (f32r)
  - entity shard: per-row norms (ACT Square+accum), normalize (DVE),
    PE-transpose -> entT[2][128,6272] (f32r)
  - big matmul: sim[128tok,448ent] tiles = projT.T @ entT (f32r, PSUM),
    PSUM->SBUF copies alternating DVE/ACT into a [128,6272] staging row,
    one 3.2MB DMA per 128-token row
  - md head: md_logits = md_W @ hidden.T (+md_b) in [3,1024] layout,
    log_softmax over the seq dim per batch segment of 128
Host side: pad/shard entity table over cores, gather/concat similarity,
reshape md output. All arithmetic runs on device.
"""
import numpy as np

import concourse.bass as bass
import concourse.bacc as bacc
import concourse.tile as tile
from concourse import masks, mybir
from concourse.bass_utils import run_bass_kernel_spmd

F32 = mybir.dt.float32
F32R = mybir.dt.float32r
AF = mybir.ActivationFunctionType
ALU = mybir.AluOpType
AX = mybir.AxisListType

B, S, H, D, E = 8, 128, 768, 256, 50000
NCORES = 8
T = B * S                   # 1024 tokens
E_LOC = 6272                # per-core entities (padded)
E_PAD = E_LOC * NCORES      # 50176
NSL = 448                   # entity slice per matmul (14 * 448 = 6272)
NT = E_LOC // NSL           # 14
KH = H // 128               # 6
KD = D // 128               # 2
MT = T // 128               # 8
EPS2 = 1e-16                # EPS**2, clamp on squared norms

_CACHE = {}


def _build():
    nc = bacc.Bacc("TRN2", target_bir_lowering=False, debug=False)
    hid = nc.dram_tensor("hidden", [T, H], F32, kind="ExternalInput").ap()
    w_in = nc.dram_tensor("W", [D, H], F32, kind="ExternalInput").ap()
    b_in = nc.dram_tensor("b", [1, D], F32, kind="ExternalInput").ap()
    mdw_in = nc.dram_tensor("md_W", [3, H], F32, kind="ExternalInput").ap()
    mdb_in = nc.dram_tensor("md_b", [3, 1], F32, kind="ExternalInput").ap()
    ent_in = nc.dram_tensor("ent", [E_LOC, D], F32, kind="ExternalInput").ap()
    sim_out = nc.dram_tensor("sim", [T, E_LOC], F32, kind="ExternalOutput").ap()
    md_out = nc.dram_tensor("md_out", [3, T], F32, kind="ExternalOutput").ap()

    with tile.TileContext(nc) as tc:
        _body(tc, hid, w_in, b_in, mdw_in, mdb_in, ent_in, sim_out, md_out)
    nc.compile()
    return nc


def _body(tc, hid, w_in, b_in, mdw_in, mdb_in, ent_in, sim_out, md_out):
    nc = tc.nc
    ncopy = [0]

    def copy_any(dst, src):
        # alternate PSUM->SBUF copies between DVE and ACT to balance load
        if ncopy[0] % 2 == 0:
            nc.vector.tensor_copy(dst, src)
        else:
            nc.scalar.copy(dst, src)
        ncopy[0] += 1

    with (
        tc.tile_pool(name="persist", bufs=1) as pp,
        tc.tile_pool(name="pin", bufs=2) as pin,
        tc.tile_pool(name="scr", bufs=2) as scr,
        tc.tile_pool(name="stage", bufs=2) as stg,
        tc.tile_pool(name="ps_tr", bufs=2, space="PSUM") as ps_tr,
        tc.tile_pool(name="ps_proj", bufs=1, space="PSUM") as ps_proj,
        tc.tile_pool(name="ps_md", bufs=1, space="PSUM") as ps_md,
        tc.tile_pool(name="ps_mm", bufs=4, space="PSUM") as ps_mm,
    ):
        ident = pp.tile([128, 128], F32, tag="ident")
        masks.make_identity(nc, ident[:])
        ones1 = pp.tile([1, 128], F32, tag="ones1")
        nc.vector.memset(ones1[:], 1.0)

        b_sb = pp.tile([1, D], F32, tag="b")
        nc.sync.dma_start(b_sb[:], b_in)
        mdb = pp.tile([3, 1], F32, tag="mdb")
        nc.sync.dma_start(mdb[:], mdb_in)
        mdw = pp.tile([3, H], F32, tag="mdw")
        nc.sync.dma_start(mdw[:], mdw_in)

        # ---- hidden load + transpose -> hT[k] (f32r) ----
        hT = [pp.tile([128, T], F32R, tag=f"hT{k}", name=f"hT{k}") for k in range(KH)]
        hid_re = hid.rearrange("(c m p) h -> p c m h", c=4, m=2, p=128)
        for c in range(4):
            hin = pin.tile([128, 2, H], F32, tag="hin")
            nc.sync.dma_start(hin[:], hid_re[:, c])
            for m2 in range(2):
                m = c * 2 + m2
                for k in range(KH):
                    tp = ps_tr.tile([128, 128], F32, tag="tr")
                    nc.tensor.transpose(
                        tp[:], hin[:, m2, k * 128:(k + 1) * 128], ident[:]
                    )
                    copy_any(hT[k][:, m * 128:(m + 1) * 128], tp[:])

        # ---- W load + transpose -> WT[k] (f32r) ----
        WT = [pp.tile([128, D], F32R, tag=f"WT{k}", name=f"WT{k}") for k in range(KH)]
        win = pin.tile([128, 2, H], F32, tag="win")
        nc.sync.dma_start(win[:], w_in.rearrange("(u p) h -> p u h", u=2, p=128))
        for k in range(KH):
            for u in range(KD):
                tp = ps_tr.tile([128, 128], F32, tag="tr")
                nc.tensor.transpose(
                    tp[:], win[:, u, k * 128:(k + 1) * 128], ident[:]
                )
                copy_any(WT[k][:, u * 128:(u + 1) * 128], tp[:])

        # ---- md_W transpose -> mdWT[k] [128,3] (f32r) ----
        mdWT = [pp.tile([128, 3], F32R, tag=f"mdWT{k}", name=f"mdWT{k}") for k in range(KH)]
        for k in range(KH):
            tp = ps_tr.tile([128, 128], F32, tag="tr")
            nc.tensor.transpose(
                tp[:, 0:3], mdw[:, k * 128:(k + 1) * 128], ident[0:3, 0:3]
            )
            copy_any(mdWT[k][:], tp[:, 0:3])

        # ---- entity shard: load, normalize, transpose -> entT[u] (f32r) ----
        entT = [pp.tile([128, E_LOC], F32R, tag=f"entT{u}", name=f"entT{u}") for u in range(KD)]
        en2 = pp.tile([128, 49], F32, tag="en2")       # squared norms
        einv = pp.tile([128, 49], F32, tag="einv")     # 1 / max(norm, eps)
        ent_re = ent_in.rearrange("(c j p) d -> p c j d", c=7, j=7, p=128)
        for c in range(7):
            ein = pin.tile([128, 7, D], F32, tag="ein")
            nc.sync.dma_start(ein[:], ent_re[:, c])
            for j in range(7):
                n = c * 7 + j
                sq = scr.tile([128, D], F32, tag="esq")
                nc.scalar.activation(
                    sq[:], ein[:, j], AF.Square, accum_out=en2[:, n:n + 1]
                )
            cols = en2[:, c * 7:(c + 1) * 7]
            icols = einv[:, c * 7:(c + 1) * 7]
            nc.vector.tensor_scalar_max(icols, cols, EPS2)
            nc.scalar.activation(icols, icols, AF.Sqrt)
            nc.vector.reciprocal(icols, icols)
            for j in range(7):
                n = c * 7 + j
                enm = scr.tile([128, D], F32, tag="enorm")
                nc.vector.tensor_scalar_mul(enm[:], ein[:, j], einv[:, n:n + 1])
                for u in range(KD):
                    tp = ps_tr.tile([128, 128], F32, tag="tr")
                    nc.tensor.transpose(
                        tp[:], enm[:, u * 128:(u + 1) * 128], ident[:]
                    )
                    copy_any(entT[u][:, n * 128:(n + 1) * 128], tp[:])

        # ---- proj = tanh(hidden @ W.T + b), token norms ----
        proj = pp.tile([128, MT, D], F32, tag="proj")
        tn2 = pp.tile([128, MT], F32, tag="tn2")
        tinv = pp.tile([128, MT], F32, tag="tinv")
        for m in range(MT):
            pj = ps_proj.tile([128, D], F32, tag="proj")
            for k in range(KH):
                nc.tensor.matmul(
                    pj[:], hT[k][:, m * 128:(m + 1) * 128], WT[k][:],
                    start=(k == 0), stop=False,
                )
            # + b via rank-1 fp32 matmul: ones[1,128].T @ b[1,256]
            nc.tensor.matmul(pj[:], ones1[:], b_sb[:], start=False, stop=True)
            nc.scalar.activation(proj[:, m], pj[:], AF.Tanh)
            sq = scr.tile([128, D], F32, tag="psq")
            nc.scalar.activation(
                sq[:], proj[:, m], AF.Square, accum_out=tn2[:, m:m + 1]
            )
        nc.vector.tensor_scalar_max(tinv[:], tn2[:], EPS2)
        nc.scalar.activation(tinv[:], tinv[:], AF.Sqrt)
        nc.vector.reciprocal(tinv[:], tinv[:])

        projT = [pp.tile([128, T], F32R, tag=f"projT{u}", name=f"projT{u}") for u in range(KD)]
        for m in range(MT):
            pn = scr.tile([128, D], F32, tag="pnorm")
            nc.vector.tensor_scalar_mul(pn[:], proj[:, m], tinv[:, m:m + 1])
            for u in range(KD):
                tp = ps_tr.tile([128, 128], F32, tag="tr")
                nc.tensor.transpose(tp[:], pn[:, u * 128:(u + 1) * 128], ident[:])
                copy_any(projT[u][:, m * 128:(m + 1) * 128], tp[:])

        # ---- md head: logits [3, T], log_softmax over seq within batch ----
        md_sb = pp.tile([3, T], F32, tag="md_sb")
        mdo = pp.tile([3, T], F32, tag="mdo")
        for s2 in range(2):
            pm = ps_md.tile([3, 512], F32, tag="md")
            for k in range(KH):
                nc.tensor.matmul(
                    pm[:], mdWT[k][:], hT[k][:, s2 * 512:(s2 + 1) * 512],
                    start=(k == 0), stop=(k == KH - 1),
                )
            nc.scalar.activation(
                md_sb[:, s2 * 512:(s2 + 1) * 512], pm[:], AF.Identity, bias=mdb[:]
            )
        mxn = pp.tile([3, MT], F32, tag="mxn")
        lse = pp.tile([3, MT], F32, tag="lse")
        for bi in range(B):
            seg = md_sb[:, bi * 128:(bi + 1) * 128]
            nc.vector.reduce_max(
                mxn[:, bi:bi + 1], seg, axis=AX.X, negate=True
            )
            ex = scr.tile([3, 128], F32, tag="mdexp")
            nc.scalar.activation(
                ex[:], seg, AF.Exp, bias=mxn[:, bi:bi + 1],
                accum_out=lse[:, bi:bi + 1],
            )
        nc.scalar.activation(lse[:], lse[:], AF.Ln)
        for bi in range(B):
            seg = md_sb[:, bi * 128:(bi + 1) * 128]
            # out = (seg + (-max)) - ln(sum(exp))
            nc.vector.tensor_scalar(
                mdo[:, bi * 128:(bi + 1) * 128], seg,
                mxn[:, bi:bi + 1], lse[:, bi:bi + 1],
                op0=ALU.add, op1=ALU.subtract,
            )
        nc.sync.dma_start(md_out, mdo[:])

        # ---- big matmul: sim rows of 128 tokens ----
        for m in range(MT):
            row = stg.tile([128, E_LOC], F32, tag="row")
            for n in range(NT):
                ps = ps_mm.tile([128, NSL], F32, tag="mm")
                nc.tensor.matmul(
                    ps[:], projT[0][:, m * 128:(m + 1) * 128],
                    entT[0][:, n * NSL:(n + 1) * NSL],
                    start=True, stop=False,
                )
                nc.tensor.matmul(
                    ps[:], projT[1][:, m * 128:(m + 1) * 128],
                    entT[1][:, n * NSL:(n + 1) * NSL],
                    start=False, stop=True,
                )
                copy_any(row[:, n * NSL:(n + 1) * NSL], ps[:])
            nc.sync.dma_start(sim_out[m * 128:(m + 1) * 128, :], row[:])


def _get_nc():
    if "nc" not in _CACHE:
        _CACHE["nc"] = _build()
    return _CACHE["nc"]


def run(inputs, trace=False):
    hs = np.ascontiguousarray(
        np.asarray(inputs["hidden_states"], dtype=np.float32).reshape(T, H)
    )
    w = np.ascontiguousarray(np.asarray(inputs["W"], dtype=np.float32))
    b = np.ascontiguousarray(
        np.asarray(inputs["b"], dtype=np.float32).reshape(1, D)
    )
    mdw = np.ascontiguousarray(np.asarray(inputs["md_W"], dtype=np.float32))
    mdb = np.ascontiguousarray(
        np.asarray(inputs["md_b"], dtype=np.float32).reshape(3, 1)
    )
    ent = np.asarray(inputs["entity_embedding"], dtype=np.float32)
    ent_pad = np.zeros((E_PAD, D), dtype=np.float32)
    ent_pad[:E] = ent

    in_maps = [
        {
            "hidden": hs, "W": w, "b": b, "md_W": mdw, "md_b": mdb,
            "ent": np.ascontiguousarray(ent_pad[c * E_LOC:(c + 1) * E_LOC]),
        }
        for c in range(NCORES)
    ]
    nc = _get_nc()
    res = run_bass_kernel_spmd(
        nc, in_maps, core_ids=list(range(NCORES)), trace=trace
    )

    sim = np.empty((T, E), dtype=np.float32)
    for c in range(NCORES):
        lo = c * E_LOC
        hi = min((c + 1) * E_LOC, E)
        if lo >= E:
            break
        sim[:, lo:hi] = res.results[c]["sim"][:, : hi - lo]
    sim = sim.reshape(B, S, E)
    md = np.ascontiguousarray(
        res.results[0]["md_out"].reshape(3, B, S).transpose(1, 2, 0)
    )
    return (md, sim), res


def kernel(**inputs):
    out, _ = run(inputs, trace=False)
    return out
